# revision 2
# baseline (speedup 1.0000x reference)
"""Bass/Trainium2 kernel for BertLikeSelfAttention (tanh softcap + ReLU-softmax).

Sharding: tensor-parallel across heads. 16 heads / 8 cores = 2 heads per core.
Each core computes its 128 output channels; host concatenates.

v2 design (vs the fp32r baseline):
  - All matmul operands in bf16 (PSUM accumulation stays fp32): same PE
    streaming rate as fp32r, but halves X DMA + SBUF, enables FWL weight
    loads, and gives DVE its 4x bf16 SBUF mode for the relu.
  - The score scale 1/(sqrt(64)*30) is folded into Wq on the host, so the
    ScalarE tanh runs with scale=1 and bias=mask/30.
  - Layouts (no on-chip transposes):
      X pre-transposed on host per batch: xt[b] = X[b].T  -> [HID, S], bf16.
      Q.T/K.T projections -> [o=128, s] (head h in partitions 64h..64h+63).
      V natural [s, o] with a ones column per head -> V_aug [s, 130]; the
      context matmul ctxT = V_aug.T @ T carries row 64 = sum_k T[k,q]
      (ReLU-softmax denominators for free).
      Scores transposed: T[k, q] = K @ Q.T (contract d=64).
  - tanh+mask fused in one ScalarE activation per [128,1024] score tile;
    relu in-place on DVE (bf16 SBUF 4x mode).
  - Context PSUM is evicted to SBUF immediately after accumulation so the
    PSUM slots free fast (no qg-boundary stall); normalization (eps +
    reciprocal of the sums row, DMA hop to partition 0, gpsimd
    partition-broadcast, multiply) runs on the SBUF copy in the shadow of
    the next qg.
  - X tiles double-buffered across batches (bufs=16) so batch b+1's
    projections overlap batch b's ACT-paced attention phase.
"""

import math
from contextlib import ExitStack

import numpy as np

import concourse.bacc as bacc
import concourse.mybir as mybir
import concourse.tile as tile
from concourse.bass_utils import run_bass_kernel_spmd

B, S, HID = 4, 2048, 1024
NH, HD = 16, 64
NCORES = 8
CPC = HID // NCORES  # output channels per core = 128
LOGITS_CAP = 30.0
EPS = 1e-6
SCALE = 1.0 / (math.sqrt(HD) * LOGITS_CAP)  # folded into Wq on host
EPS_ADJ = EPS / LOGITS_CAP

F32 = mybir.dt.float32
BF16 = mybir.dt.bfloat16

NKT = S // 128  # 16 key tiles
NQG = S // 512  # 4 query groups
NHT = HID // 128  # 8 hidden (contraction) tiles


def build_program(reps=1):
    import contextlib
    nc = bacc.Bacc("TRN2", target_bir_lowering=False, debug=False)

    xt_d = nc.dram_tensor("xt", [B, HID, S], BF16, kind="ExternalInput")
    wqt_d = nc.dram_tensor("wqt", [HID, CPC], BF16, kind="ExternalInput")
    wkt_d = nc.dram_tensor("wkt", [HID, CPC], BF16, kind="ExternalInput")
    wvt_d = nc.dram_tensor("wvt", [HID, CPC], BF16, kind="ExternalInput")
    bq_d = nc.dram_tensor("bqv", [CPC, 1], F32, kind="ExternalInput")
    bk_d = nc.dram_tensor("bkv", [CPC, 1], F32, kind="ExternalInput")
    bvb_d = nc.dram_tensor("bvb", [128, CPC], F32, kind="ExternalInput")
    mask_d = nc.dram_tensor("maskd", [B, S], F32, kind="ExternalInput")
    out_d = nc.dram_tensor("out_t", [B, 2, HD, S], F32, kind="ExternalOutput")

    TANH = mybir.ActivationFunctionType.Tanh

    with tile.TileContext(nc) as tc, ExitStack() as ctx:
        consts = ctx.enter_context(tc.tile_pool(name="consts", bufs=1))
        xt_pool = ctx.enter_context(tc.tile_pool(name="xtp", bufs=16))
        qk_pool = ctx.enter_context(tc.tile_pool(name="qkp", bufs=2))
        v_pool = ctx.enter_context(tc.tile_pool(name="vp", bufs=32))
        tt_pool = ctx.enter_context(tc.tile_pool(name="ttp", bufs=4))
        sm_pool = ctx.enter_context(tc.tile_pool(name="smp", bufs=2))
        ob_pool = ctx.enter_context(tc.tile_pool(name="obp", bufs=4))
        pproj = ctx.enter_context(tc.tile_pool(name="pproj", bufs=2, space="PSUM"))
        psc = ctx.enter_context(tc.tile_pool(name="psc", bufs=2, space="PSUM"))
        pctx = ctx.enter_context(tc.tile_pool(name="pctx", bufs=2, space="PSUM"))

        # --- constants ---
        wq_sb = consts.tile([128, NHT, 128], BF16, name="wq_sb")
        wk_sb = consts.tile([128, NHT, 128], BF16, name="wk_sb")
        wv_sb = consts.tile([128, NHT, 128], BF16, name="wv_sb")
        nc.sync.dma_start(wq_sb, wqt_d.rearrange("(j p) o -> p j o", p=128))
        nc.sync.dma_start(wk_sb, wkt_d.rearrange("(j p) o -> p j o", p=128))
        nc.sync.dma_start(wv_sb, wvt_d.rearrange("(j p) o -> p j o", p=128))
        bq_sb = consts.tile([CPC, 1], F32, name="bq_sb")
        bk_sb = consts.tile([CPC, 1], F32, name="bk_sb")
        bvb_sb = consts.tile([128, CPC], F32, name="bvb_sb")
        nc.sync.dma_start(bq_sb, bq_d[:, :])
        nc.sync.dma_start(bk_sb, bk_d[:, :])
        nc.sync.dma_start(bvb_sb, bvb_d[:, :])
        mask_sb = consts.tile([128, B, NKT], F32, name="mask_sb")
        nc.sync.dma_start(mask_sb, mask_d.rearrange("b (k p) -> p b k", p=128))

        loop_cm = tc.For_i(0, reps, 1) if reps > 1 else contextlib.nullcontext()
        with loop_cm:
          for b in range(B):
            # --- load X.T tiles for this batch (bufs=16: b+1 prefetches) ---
            xts = []
            for j in range(NHT):
                xtile = xt_pool.tile([128, S], BF16, name=f"xt_{b}_{j}", tag="xt")
                nc.sync.dma_start(xtile, xt_d[b, j * 128 : (j + 1) * 128, :])
                xts.append(xtile)

            # --- Q.T / K.T projections: out [o=128, s], bf16 ---
            qt = qk_pool.tile([128, S], BF16, name=f"qt_{b}", tag="qt")
            kt = qk_pool.tile([128, S], BF16, name=f"kt_{b}", tag="kt")
            for dst, w_sb, b_sb in ((qt, wq_sb, bq_sb), (kt, wk_sb, bk_sb)):
                for sg in range(NQG):
                    ps = pproj.tile([128, 512], F32, name=f"psq_{b}_{sg}", tag="proj")
                    for j in range(NHT):
                        nc.tensor.matmul(
                            ps,
                            w_sb[:, j, :],
                            xts[j][:, sg * 512 : (sg + 1) * 512],
                            start=(j == 0),
                            stop=(j == NHT - 1),
                        )
                    nc.vector.tensor_scalar_add(
                        dst[:, sg * 512 : (sg + 1) * 512], ps, b_sb
                    )

            # --- V projection, natural layout [s, o], ones columns, bf16 ---
            vs = []
            for st in range(NKT):
                ps = pproj.tile([128, 128], F32, name=f"psv_{b}_{st}", tag="proj")
                for j in range(NHT):
                    nc.tensor.matmul(
                        ps,
                        xts[j][:, st * 128 : (st + 1) * 128],
                        wv_sb[:, j, :],
                        start=(j == 0),
                        stop=(j == NHT - 1),
                    )
                v = v_pool.tile([128, 130], BF16, name=f"v_{b}_{st}", tag="v")
                nc.vector.tensor_add(v[:, 0:64], ps[:, 0:64], bvb_sb[:, 0:64])
                nc.vector.tensor_add(v[:, 65:129], ps[:, 64:128], bvb_sb[:, 64:128])
                # ones columns (out = in*0 + 1)
                nc.vector.tensor_scalar(
                    v[:, 64:65], bvb_sb[:, 0:1], 0.0, 1.0,
                    mybir.AluOpType.mult, mybir.AluOpType.add,
                )
                nc.vector.tensor_scalar(
                    v[:, 129:130], bvb_sb[:, 0:1], 0.0, 1.0,
                    mybir.AluOpType.mult, mybir.AluOpType.add,
                )
                vs.append(v)

            # --- attention ---
            for qg in range(NQG):
                q0 = qg * 512
                cA = pctx.tile([65, 512], F32, name=f"cA_{b}_{qg}", tag="ctx")
                cB = pctx.tile([65, 512], F32, name=f"cB_{b}_{qg}", tag="ctx")
                for kb in range(NKT):
                    k0 = kb * 128
                    sps = psc.tile([128, 1024], F32, name=f"sps_{b}_{qg}_{kb}", tag="sc")
                    # transposed scores T[k, q] per head
                    nc.tensor.matmul(
                        sps[:, 0:512],
                        kt[0:64, k0 : k0 + 128],
                        qt[0:64, q0 : q0 + 512],
                        start=True,
                        stop=True,
                    )
                    nc.tensor.matmul(
                        sps[:, 512:1024],
                        kt[64:128, k0 : k0 + 128],
                        qt[64:128, q0 : q0 + 512],
                        start=True,
                        stop=True,
                    )
                    ttile = tt_pool.tile([128, 1024], BF16, name=f"tt_{b}_{qg}_{kb}", tag="tt")
                    nc.scalar.activation(
                        ttile, sps, TANH, bias=mask_sb[:, b, kb : kb + 1], scale=1.0
                    )
                    nc.vector.tensor_scalar_max(ttile, ttile, 0.0)
                    nc.tensor.matmul(
                        cA,
                        vs[kb][:, 0:65],
                        ttile[:, 0:512],
                        start=(kb == 0),
                        stop=(kb == NKT - 1),
                    )
                    nc.tensor.matmul(
                        cB,
                        vs[kb][:, 65:130],
                        ttile[:, 512:1024],
                        start=(kb == 0),
                        stop=(kb == NKT - 1),
                    )

                # --- evict ctx PSUM to SBUF fast, then normalize there ---
                sA = sm_pool.tile([65, 512], F32, name=f"sA_{b}_{qg}", tag="sA")
                sB = sm_pool.tile([65, 512], F32, name=f"sB_{b}_{qg}", tag="sB")
                nc.vector.tensor_copy(sA, cA)
                nc.vector.tensor_copy(sB, cB)
                # eps + reciprocal on the sums row (partition 64)
                nc.vector.tensor_scalar_add(sA[64:65, :], sA[64:65, :], EPS_ADJ)
                nc.vector.tensor_scalar_add(sB[64:65, :], sB[64:65, :], EPS_ADJ)
                nc.vector.reciprocal(sA[64:65, :], sA[64:65, :])
                nc.vector.reciprocal(sB[64:65, :], sB[64:65, :])
                # DMA hop to partition 0 (gpsimd broadcast reads partition 0)
                hopA = sm_pool.tile([1, 512], F32, name=f"hopA_{b}_{qg}", tag="hopA")
                hopB = sm_pool.tile([1, 512], F32, name=f"hopB_{b}_{qg}", tag="hopB")
                nc.sync.dma_start(hopA, sA[64:65, :])
                nc.sync.dma_start(hopB, sB[64:65, :])
                rbA = sm_pool.tile([64, 512], F32, name=f"rbA_{b}_{qg}", tag="rbA")
                rbB = sm_pool.tile([64, 512], F32, name=f"rbB_{b}_{qg}", tag="rbB")
                nc.gpsimd.partition_broadcast(rbA, hopA, channels=64)
                nc.gpsimd.partition_broadcast(rbB, hopB, channels=64)
                obA = ob_pool.tile([64, 512], F32, name=f"obA_{b}_{qg}", tag="obA")
                obB = ob_pool.tile([64, 512], F32, name=f"obB_{b}_{qg}", tag="obB")
                nc.vector.tensor_mul(obA, sA[0:64, :], rbA)
                nc.vector.tensor_mul(obB, sB[0:64, :], rbB)
                nc.sync.dma_start(out_d[b, 0, :, q0 : q0 + 512], obA)
                nc.sync.dma_start(out_d[b, 1, :, q0 : q0 + 512], obB)

    nc.compile()
    return nc


_CACHE = {}


def _get_nc():
    if "nc" not in _CACHE:
        _CACHE["nc"] = build_program()
    return _CACHE["nc"]


def make_in_maps(inputs):
    """Build the 8 per-core input maps from full-size numpy inputs."""
    import ml_dtypes

    hidden_states = np.asarray(inputs["hidden_states"], dtype=np.float32)
    attention_mask = np.asarray(inputs["attention_mask"], dtype=np.float32)
    Wq = np.asarray(inputs["Wq"], dtype=np.float32)
    Wk = np.asarray(inputs["Wk"], dtype=np.float32)
    Wv = np.asarray(inputs["Wv"], dtype=np.float32)
    bq = np.asarray(inputs["bq"], dtype=np.float32)
    bk = np.asarray(inputs["bk"], dtype=np.float32)
    bv = np.asarray(inputs["bv"], dtype=np.float32)

    xt = np.ascontiguousarray(
        hidden_states.transpose(0, 2, 1)
    ).astype(ml_dtypes.bfloat16)  # [B, HID, S] bf16
    maskd = np.ascontiguousarray(
        attention_mask.reshape(B, S) / np.float32(LOGITS_CAP)
    )

    in_maps = []
    for i in range(NCORES):
        lo, hi = i * CPC, (i + 1) * CPC
        in_maps.append(
            {
                "xt": xt,
                # score scale folded into Wq (and its bias)
                "wqt": np.ascontiguousarray(
                    (Wq[lo:hi, :] * np.float32(SCALE)).T
                ).astype(ml_dtypes.bfloat16),
                "wkt": np.ascontiguousarray(Wk[lo:hi, :].T).astype(
                    ml_dtypes.bfloat16
                ),
                "wvt": np.ascontiguousarray(Wv[lo:hi, :].T).astype(
                    ml_dtypes.bfloat16
                ),
                "bqv": np.ascontiguousarray(
                    (bq[lo:hi] * np.float32(SCALE)).reshape(CPC, 1)
                ),
                "bkv": np.ascontiguousarray(bk[lo:hi].reshape(CPC, 1)),
                "bvb": np.ascontiguousarray(
                    np.tile(bv[lo:hi][None, :], (128, 1))
                ),
                "maskd": maskd,
            }
        )
    return in_maps


def kernel(hidden_states, attention_mask, Wq, bq, Wk, bk, Wv, bv):
    nc = _get_nc()
    in_maps = make_in_maps(
        {
            "hidden_states": hidden_states,
            "attention_mask": attention_mask,
            "Wq": Wq,
            "bq": bq,
            "Wk": Wk,
            "bk": bk,
            "Wv": Wv,
            "bv": bv,
        }
    )

    res = None
    last_err = None
    for attempt in range(3):
        try:
            res = run_bass_kernel_spmd(nc, in_maps, list(range(NCORES)))
            break
        except Exception as e:  # transient NRT/axon device errors: retry
            last_err = e
            import time as _time

            _time.sleep(2.0 * (attempt + 1))
    if res is None:
        raise last_err

    out = np.empty((B, S, HID), dtype=np.float32)
    for i in range(NCORES):
        o = res.results[i]["out_t"]  # [B, 2, HD, S]
        out[:, :, i * CPC : (i + 1) * CPC] = (
            o.transpose(0, 3, 1, 2).reshape(B, S, CPC)
        )
    return out


# revision 5
# speedup vs baseline: 4.1407x; 4.1407x over previous
"""Bass/Trainium2 kernel for BertLikeSelfAttention (tanh softcap + ReLU-softmax).

Sharding: tensor-parallel across heads. 16 heads / 8 cores = 2 heads per core.
Each core computes its 128 output channels; host concatenates.

v2 design (vs the fp32r baseline):
  - All matmul operands in bf16 (PSUM accumulation stays fp32): same PE
    streaming rate as fp32r, but halves X DMA + SBUF, enables FWL weight
    loads, and gives DVE its 4x bf16 SBUF mode for the relu.
  - The score scale 1/(sqrt(64)*30) is folded into Wq on the host, so the
    ScalarE tanh runs with scale=1 and bias=mask/30.
  - Layouts (no on-chip transposes):
      X pre-transposed on host per batch: xt[b] = X[b].T  -> [HID, S], bf16.
      Q.T/K.T projections -> [o=128, s] (head h in partitions 64h..64h+63).
      V natural [s, o] with a ones column per head -> V_aug [s, 130]; the
      context matmul ctxT = V_aug.T @ T carries row 64 = sum_k T[k,q]
      (ReLU-softmax denominators for free).
      Scores transposed: T[k, q] = K @ Q.T (contract d=64).
  - tanh+mask fused in one ScalarE activation per [128,1024] score tile;
    relu in-place on DVE (bf16 SBUF 4x mode).
  - Context PSUM is evicted to SBUF immediately after accumulation so the
    PSUM slots free fast (no qg-boundary stall); normalization (eps +
    reciprocal of the sums row, DMA hop to partition 0, gpsimd
    partition-broadcast, multiply) runs on the SBUF copy in the shadow of
    the next qg.
  - X tiles double-buffered across batches (bufs=16) so batch b+1's
    projections overlap batch b's ACT-paced attention phase.
"""

import math
from contextlib import ExitStack

import numpy as np

import concourse.bacc as bacc
import concourse.mybir as mybir
import concourse.tile as tile
from concourse.bass_utils import run_bass_kernel_spmd

B, S, HID = 4, 2048, 1024
NH, HD = 16, 64
NCORES = 8
CPC = HID // NCORES  # output channels per core = 128
LOGITS_CAP = 30.0
EPS = 1e-6
SCALE = 1.0 / (math.sqrt(HD) * LOGITS_CAP)  # folded into Wq on host
EPS_ADJ = EPS / LOGITS_CAP

F32 = mybir.dt.float32
BF16 = mybir.dt.bfloat16

NKT = S // 128  # 16 key tiles
NQG = S // 512  # 4 query groups
NHT = HID // 128  # 8 hidden (contraction) tiles


def build_program(reps=1, phase="full"):
    # phase: "full" | "nonorm" | "noctx" | "noact" | "proj" -- truncated
    # builds for phase-delta profiling (each adds a cheap sink so nothing
    # is dead-code eliminated)
    import contextlib
    nc = bacc.Bacc("TRN2", target_bir_lowering=False, debug=False)

    xt_d = nc.dram_tensor("xt", [B, HID, S], BF16, kind="ExternalInput")
    wqt_d = nc.dram_tensor("wqt", [HID, CPC], BF16, kind="ExternalInput")
    wkt_d = nc.dram_tensor("wkt", [HID, CPC], BF16, kind="ExternalInput")
    wvt_d = nc.dram_tensor("wvt", [HID, CPC], BF16, kind="ExternalInput")
    bq_d = nc.dram_tensor("bqv", [CPC, 1], F32, kind="ExternalInput")
    bk_d = nc.dram_tensor("bkv", [CPC, 1], F32, kind="ExternalInput")
    bvb_d = nc.dram_tensor("bvb", [128, CPC], F32, kind="ExternalInput")
    mask_d = nc.dram_tensor("maskd", [B, S], F32, kind="ExternalInput")
    out_d = nc.dram_tensor("out_t", [B, 2, HD, S], F32, kind="ExternalOutput")

    TANH = mybir.ActivationFunctionType.Tanh

    with tile.TileContext(nc) as tc, ExitStack() as ctx:
        consts = ctx.enter_context(tc.tile_pool(name="consts", bufs=1))
        xt_pool = ctx.enter_context(tc.tile_pool(name="xtp", bufs=16))
        qk_pool = ctx.enter_context(tc.tile_pool(name="qkp", bufs=2))
        v_pool = ctx.enter_context(tc.tile_pool(name="vp", bufs=32))
        tt_pool = ctx.enter_context(tc.tile_pool(name="ttp", bufs=4))
        sm_pool = ctx.enter_context(tc.tile_pool(name="smp", bufs=2))
        ob_pool = ctx.enter_context(tc.tile_pool(name="obp", bufs=4))
        pproj = ctx.enter_context(tc.tile_pool(name="pproj", bufs=2, space="PSUM"))
        psc = ctx.enter_context(tc.tile_pool(name="psc", bufs=2, space="PSUM"))
        pctx = ctx.enter_context(tc.tile_pool(name="pctx", bufs=2, space="PSUM"))

        # --- constants ---
        wq_sb = consts.tile([128, NHT, 128], BF16, name="wq_sb")
        wk_sb = consts.tile([128, NHT, 128], BF16, name="wk_sb")
        wv_sb = consts.tile([128, NHT, 128], BF16, name="wv_sb")
        nc.sync.dma_start(wq_sb, wqt_d.rearrange("(j p) o -> p j o", p=128))
        nc.sync.dma_start(wk_sb, wkt_d.rearrange("(j p) o -> p j o", p=128))
        nc.sync.dma_start(wv_sb, wvt_d.rearrange("(j p) o -> p j o", p=128))
        bq_sb = consts.tile([CPC, 1], F32, name="bq_sb")
        bk_sb = consts.tile([CPC, 1], F32, name="bk_sb")
        bvb_sb = consts.tile([128, CPC], F32, name="bvb_sb")
        nc.sync.dma_start(bq_sb, bq_d[:, :])
        nc.sync.dma_start(bk_sb, bk_d[:, :])
        nc.sync.dma_start(bvb_sb, bvb_d[:, :])
        mask_sb = consts.tile([128, B, NKT], F32, name="mask_sb")
        nc.sync.dma_start(mask_sb, mask_d.rearrange("b (k p) -> p b k", p=128))

        loop_cm = (tc.For_i(0, reps, 1, hint_engines=(mybir.EngineType.PE, mybir.EngineType.DVE, mybir.EngineType.Activation)) if reps > 1 else contextlib.nullcontext())
        with loop_cm:
          for b in range(B):
            # --- load X.T tiles for this batch (bufs=16: b+1 prefetches) ---
            xts = []
            for j in range(NHT):
                xtile = xt_pool.tile([128, S], BF16, name=f"xt_{b}_{j}", tag="xt")
                nc.sync.dma_start(xtile, xt_d[b, j * 128 : (j + 1) * 128, :])
                xts.append(xtile)

            # --- Q.T / K.T projections: out [o=128, s], bf16 ---
            qt = qk_pool.tile([128, S], BF16, name=f"qt_{b}", tag="qt")
            kt = qk_pool.tile([128, S], BF16, name=f"kt_{b}", tag="kt")
            for dst, w_sb, b_sb in ((qt, wq_sb, bq_sb), (kt, wk_sb, bk_sb)):
                for sg in range(NQG):
                    ps = pproj.tile([128, 512], F32, name=f"psq_{b}_{sg}", tag="proj")
                    for j in range(NHT):
                        nc.tensor.matmul(
                            ps,
                            w_sb[:, j, :],
                            xts[j][:, sg * 512 : (sg + 1) * 512],
                            start=(j == 0),
                            stop=(j == NHT - 1),
                        )
                    nc.vector.tensor_scalar_add(
                        dst[:, sg * 512 : (sg + 1) * 512], ps, b_sb
                    )

            # --- V projection, natural layout [s, o], ones columns, bf16 ---
            vs = []
            for st in range(NKT):
                ps = pproj.tile([128, 128], F32, name=f"psv_{b}_{st}", tag="proj")
                for j in range(NHT):
                    nc.tensor.matmul(
                        ps,
                        xts[j][:, st * 128 : (st + 1) * 128],
                        wv_sb[:, j, :],
                        start=(j == 0),
                        stop=(j == NHT - 1),
                    )
                v = v_pool.tile([128, 130], BF16, name=f"v_{b}_{st}", tag="v")
                nc.vector.tensor_add(v[:, 0:64], ps[:, 0:64], bvb_sb[:, 0:64])
                nc.vector.tensor_add(v[:, 65:129], ps[:, 64:128], bvb_sb[:, 64:128])
                # ones columns (out = in*0 + 1): ctx matmul row 64 = sums
                nc.vector.tensor_scalar(
                    v[:, 64:65], bvb_sb[:, 0:1], 0.0, 1.0,
                    mybir.AluOpType.mult, mybir.AluOpType.add,
                )
                nc.vector.tensor_scalar(
                    v[:, 129:130], bvb_sb[:, 0:1], 0.0, 1.0,
                    mybir.AluOpType.mult, mybir.AluOpType.add,
                )
                vs.append(v)

            # --- phase-truncated sinks ---
            if phase == "proj":
                snk = ob_pool.tile([128, 512], F32, name=f"snk_{b}", tag="snk")
                nc.vector.tensor_copy(snk[:, 0:512], qt[:, 0:512])
                nc.vector.tensor_copy(snk[:, 0:512], kt[:, 0:512])
                for st in range(NKT):
                    nc.vector.tensor_copy(snk[:, 0:130], vs[st][:, :])
                nc.gpsimd.dma_start(out_d[b, 0, :, 0:512], snk[0:64, :])
                continue

            # --- attention ---
            for qg in range(NQG):
                q0 = qg * 512
                if phase in ("full", "nonorm"):
                    cA = pctx.tile([65, 512], F32, name=f"cA_{b}_{qg}", tag="ctx")
                    cB = pctx.tile([65, 512], F32, name=f"cB_{b}_{qg}", tag="ctx")
                else:
                    cA = pctx.tile([1, 64], F32, name=f"cA_{b}_{qg}", tag="ctx")
                    cB = pctx.tile([1, 64], F32, name=f"cB_{b}_{qg}", tag="ctx")
                for kb in range(NKT):
                    k0 = kb * 128
                    sps = psc.tile([128, 1024], F32, name=f"sps_{b}_{qg}_{kb}", tag="sc")
                    # transposed scores T[k, q] per head
                    nc.tensor.matmul(
                        sps[:, 0:512],
                        kt[0:64, k0 : k0 + 128],
                        qt[0:64, q0 : q0 + 512],
                        start=True,
                        stop=True,
                    )
                    nc.tensor.matmul(
                        sps[:, 512:1024],
                        kt[64:128, k0 : k0 + 128],
                        qt[64:128, q0 : q0 + 512],
                        start=True,
                        stop=True,
                    )
                    if phase == "noact":
                        # cheap DVE sink on a PSUM slice
                        snk2 = ob_pool.tile([128, 64], BF16, name=f"sk_{b}_{qg}_{kb}", tag="sk2")
                        nc.vector.tensor_scalar_max(snk2, sps[:, 0:64], 0.0)
                        continue
                    ttile = tt_pool.tile([128, 1024], BF16, name=f"tt_{b}_{qg}_{kb}", tag="tt")
                    nc.scalar.activation(
                        ttile, sps, TANH, bias=mask_sb[:, b, kb : kb + 1], scale=1.0
                    )
                    nc.vector.tensor_scalar_max(ttile, ttile, 0.0)
                    if phase == "noctx":
                        # cheap PE sink: 1-row matmul over a 64-wide slice
                        nc.tensor.matmul(
                            cA, vs[kb][:, 64:65], ttile[:, 0:64],
                            start=(kb == 0), stop=(kb == NKT - 1),
                        )
                        continue
                    nc.tensor.matmul(
                        cA,
                        vs[kb][:, 0:65],
                        ttile[:, 0:512],
                        start=(kb == 0),
                        stop=(kb == NKT - 1),
                    )
                    nc.tensor.matmul(
                        cB,
                        vs[kb][:, 65:130],
                        ttile[:, 512:1024],
                        start=(kb == 0),
                        stop=(kb == NKT - 1),
                    )

                if phase in ("noctx", "noact"):
                    so = ob_pool.tile([1, 64], F32, name=f"so_{b}_{qg}", tag="so")
                    nc.vector.tensor_copy(so, cA)
                    nc.gpsimd.dma_start(out_d[b, 0, 0:1, q0 : q0 + 64], so)
                    continue
                # --- evict ctx PSUM to SBUF fast, then normalize there ---
                sA = sm_pool.tile([65, 512], F32, name=f"sA_{b}_{qg}", tag="sA")
                sB = sm_pool.tile([65, 512], F32, name=f"sB_{b}_{qg}", tag="sB")
                nc.vector.tensor_copy(sA, cA)
                nc.vector.tensor_copy(sB, cB)
                if phase == "nonorm":
                    nc.gpsimd.dma_start(out_d[b, 0, :, q0 : q0 + 512], sA[0:64, :])
                    nc.gpsimd.dma_start(out_d[b, 1, :, q0 : q0 + 512], sB[0:64, :])
                    continue
                # eps + reciprocal on the sums row (partition 64)
                nc.vector.tensor_scalar_add(sA[64:65, :], sA[64:65, :], EPS_ADJ)
                nc.vector.tensor_scalar_add(sB[64:65, :], sB[64:65, :], EPS_ADJ)
                nc.vector.reciprocal(sA[64:65, :], sA[64:65, :])
                nc.vector.reciprocal(sB[64:65, :], sB[64:65, :])
                # DMA hop to partition 0 (gpsimd broadcast reads partition 0;
                # DVE partition ranges must be 0-aligned) -- issued on the
                # gpsimd queue to keep SP free
                hopA = sm_pool.tile([1, 512], F32, name=f"hopA_{b}_{qg}", tag="hopA")
                hopB = sm_pool.tile([1, 512], F32, name=f"hopB_{b}_{qg}", tag="hopB")
                nc.gpsimd.dma_start(hopA, sA[64:65, :])
                nc.gpsimd.dma_start(hopB, sB[64:65, :])
                rbA = sm_pool.tile([64, 512], F32, name=f"rbA_{b}_{qg}", tag="rbA")
                rbB = sm_pool.tile([64, 512], F32, name=f"rbB_{b}_{qg}", tag="rbB")
                nc.gpsimd.partition_broadcast(rbA, hopA, channels=64)
                nc.gpsimd.partition_broadcast(rbB, hopB, channels=64)
                obA = ob_pool.tile([64, 512], F32, name=f"obA_{b}_{qg}", tag="obA")
                obB = ob_pool.tile([64, 512], F32, name=f"obB_{b}_{qg}", tag="obB")
                nc.vector.tensor_mul(obA, sA[0:64, :], rbA)
                nc.vector.tensor_mul(obB, sB[0:64, :], rbB)
                nc.gpsimd.dma_start(out_d[b, 0, :, q0 : q0 + 512], obA)
                nc.gpsimd.dma_start(out_d[b, 1, :, q0 : q0 + 512], obB)

    nc.compile()
    return nc


_CACHE = {}


def _get_nc():
    if "nc" not in _CACHE:
        _CACHE["nc"] = build_program()
    return _CACHE["nc"]


def make_in_maps(inputs):
    """Build the 8 per-core input maps from full-size numpy inputs."""
    import ml_dtypes

    hidden_states = np.asarray(inputs["hidden_states"], dtype=np.float32)
    attention_mask = np.asarray(inputs["attention_mask"], dtype=np.float32)
    Wq = np.asarray(inputs["Wq"], dtype=np.float32)
    Wk = np.asarray(inputs["Wk"], dtype=np.float32)
    Wv = np.asarray(inputs["Wv"], dtype=np.float32)
    bq = np.asarray(inputs["bq"], dtype=np.float32)
    bk = np.asarray(inputs["bk"], dtype=np.float32)
    bv = np.asarray(inputs["bv"], dtype=np.float32)

    xt = np.ascontiguousarray(
        hidden_states.transpose(0, 2, 1)
    ).astype(ml_dtypes.bfloat16)  # [B, HID, S] bf16
    maskd = np.ascontiguousarray(
        attention_mask.reshape(B, S) / np.float32(LOGITS_CAP)
    )

    in_maps = []
    for i in range(NCORES):
        lo, hi = i * CPC, (i + 1) * CPC
        in_maps.append(
            {
                "xt": xt,
                # score scale folded into Wq (and its bias)
                "wqt": np.ascontiguousarray(
                    (Wq[lo:hi, :] * np.float32(SCALE)).T
                ).astype(ml_dtypes.bfloat16),
                "wkt": np.ascontiguousarray(Wk[lo:hi, :].T).astype(
                    ml_dtypes.bfloat16
                ),
                "wvt": np.ascontiguousarray(Wv[lo:hi, :].T).astype(
                    ml_dtypes.bfloat16
                ),
                "bqv": np.ascontiguousarray(
                    (bq[lo:hi] * np.float32(SCALE)).reshape(CPC, 1)
                ),
                "bkv": np.ascontiguousarray(bk[lo:hi].reshape(CPC, 1)),
                "bvb": np.ascontiguousarray(
                    np.tile(bv[lo:hi][None, :], (128, 1))
                ),
                "maskd": maskd,
            }
        )
    return in_maps


def kernel(hidden_states, attention_mask, Wq, bq, Wk, bk, Wv, bv):
    nc = _get_nc()
    in_maps = make_in_maps(
        {
            "hidden_states": hidden_states,
            "attention_mask": attention_mask,
            "Wq": Wq,
            "bq": bq,
            "Wk": Wk,
            "bk": bk,
            "Wv": Wv,
            "bv": bv,
        }
    )

    res = None
    last_err = None
    for attempt in range(3):
        try:
            res = run_bass_kernel_spmd(nc, in_maps, list(range(NCORES)))
            break
        except Exception as e:  # transient NRT/axon device errors: retry
            last_err = e
            import time as _time

            _time.sleep(2.0 * (attempt + 1))
    if res is None:
        raise last_err

    out = np.empty((B, S, HID), dtype=np.float32)
    for i in range(NCORES):
        o = res.results[i]["out_t"]  # [B, 2, HD, S]
        out[:, :, i * CPC : (i + 1) * CPC] = (
            o.transpose(0, 3, 1, 2).reshape(B, S, CPC)
        )
    return out
